# revision 28
# baseline (speedup 1.0000x reference)
"""Trainium2 Bass kernel for nn_DataONEEncoder (2-layer GRU + LN + pool + proj + GELU).

Data-parallel over batch: B=256 -> 32 per core on 8 NeuronCores, no collectives.

v2: single software-pipelined main loop. Layer-0 scan steps, layer-1 scan steps
(lagging 2 groups of 8 steps), and the gx0/gx1 input-projection GEMMs (computed
as j-slices sprinkled between scan steps) all interleave so the PE never idles
long enough to stall on the serial gate chain or re-throttle (HAM). gx and h
intermediates live in SBUF rings; only xm (in) and h2 (out, for the LN phase)
touch DRAM.

Phase E (LayerNorm + pool + proj + GELU): LN affine (g, b) is folded into
W_proj/b_proj on the host; rsqrt computed as exp(-0.5*ln(var+eps)) on the
scalar engine with 128-partition-parallel stats (mean via ones/H matmul
broadcast), avoiding the 1-partition DVE reciprocal.
"""

import os
import numpy as np
import ml_dtypes

import concourse.bass as bass
from concourse import bacc
import concourse.mybir as mybir
import concourse.tile as tile
from concourse.alu_op_type import AluOpType
from concourse.bass import ts, ds

B, T, F, H = 256, 512, 65, 512
NCORES = 8
BL = B // NCORES          # 32 batch per core
H3 = 3 * H                # 1536
NJ = H3 // 128            # 12 output tiles of the gate dim
NK = H // 128             # 4 contraction tiles of the hidden dim
TB = T * BL               # tokens per core
EPS = 1e-5
GROUP = 8                 # scan steps per pipeline group
NG = T // GROUP           # 64 groups
CHTOK = GROUP * BL        # 256 tokens per gx GEMM chunk

f32 = mybir.dt.float32
f32r = mybir.dt.float32r
bf16 = mybir.dt.bfloat16
AF = mybir.ActivationFunctionType

USE_HILO = os.environ.get("KERNEL_HILO", "0") == "1"
SIM_MODE = os.environ.get("KERNEL_SIM", "0") == "1"   # CoreSim lacks Gelu


def r32(ap):
    return ap.bitcast(f32r)


def build_nc():
    nc = bacc.Bacc()

    # ---- external inputs (host pre-laid-out, see kernel()) ----
    xmT = nc.declare_dram_parameter("xmT", [2 * F, TB], f32r, isOutput=False)    # [f, (t,b)]
    w0T = nc.declare_dram_parameter("w0T", [F, 2, H3], f32r, isOutput=False)     # [f, k(x|m), g]
    w1T = nc.declare_dram_parameter("w1T", [128, NK, H3], f32r, isOutput=False)  # [p, k, g]
    whh0 = nc.declare_dram_parameter("whh0", [128, NJ, NK, 128], bf16, isOutput=False)
    whh1 = nc.declare_dram_parameter("whh1", [128, NJ, NK, 128], bf16, isOutput=False)
    gb0 = nc.declare_dram_parameter("gb0", [128, NJ], f32, isOutput=False)       # folded bias
    gb1 = nc.declare_dram_parameter("gb1", [128, NJ], f32, isOutput=False)
    bhn0 = nc.declare_dram_parameter("bhn0", [128, NK], f32, isOutput=False)     # b_hh n-gate
    bhn1 = nc.declare_dram_parameter("bhn1", [128, NK], f32, isOutput=False)
    wpT = nc.declare_dram_parameter("wpT", [128, NK, 256], f32r, isOutput=False)  # ln_g folded
    bp = nc.declare_dram_parameter("bp", [128, 2], f32, isOutput=False)           # 2*Wp@ln_b folded
    out = nc.declare_dram_parameter("out", [2, 128, BL], f32, isOutput=True)

    with tile.TileContext(nc) as tc:
        with tc.tile_pool(name="consts", bufs=1) as consts:

            # ---- load constants to SBUF ----
            w0_sb = consts.tile([F, 2, H3], f32r)
            nc.sync.dma_start(out=w0_sb, in_=w0T[:])
            w1_sb = consts.tile([128, NK, H3], f32r)
            nc.sync.dma_start(out=w1_sb, in_=w1T[:])
            whh_sb = [consts.tile([128, NJ, NK, 128], bf16, name=f"whh{i}_sb") for i in range(2)]
            nc.sync.dma_start(out=whh_sb[0], in_=whh0[:])
            nc.sync.dma_start(out=whh_sb[1], in_=whh1[:])
            gb_sb = [consts.tile([128, NJ], f32, name=f"gb{i}_sb") for i in range(2)]
            nc.sync.dma_start(out=gb_sb[0], in_=gb0[:])
            nc.sync.dma_start(out=gb_sb[1], in_=gb1[:])
            # broadcast b_hh(n) over batch -> [128, NK, BL]
            bhn_small = [consts.tile([128, NK], f32, name=f"bhn{i}_sm") for i in range(2)]
            bhn_sb = [consts.tile([128, NK, BL], f32, name=f"bhn{i}_sb") for i in range(2)]
            for i, srcp in enumerate((bhn0, bhn1)):
                nc.sync.dma_start(out=bhn_small[i], in_=srcp[:])
                nc.vector.tensor_copy(out=bhn_sb[i],
                                      in_=bhn_small[i].to_broadcast([128, NK, BL]))
            wp_sb = consts.tile([128, NK, 256], f32r)
            nc.sync.dma_start(out=wp_sb, in_=wpT[:])
            bp_sb = consts.tile([128, 2], f32)
            nc.sync.dma_start(out=bp_sb, in_=bp[:])
            ones_stage = consts.tile([128, 128], f32)
            nc.vector.memset(ones_stage, 1.0 / H)
            onesH = consts.tile([128, 128], f32r)   # lhsT for partition-mean bcast
            nc.vector.tensor_copy(out=onesH, in_=ones_stage)
            eps_sb = consts.tile([128, 1], f32)
            nc.vector.memset(eps_sb, EPS)
            h0z = consts.tile([128, NK, BL], f32)
            nc.vector.memset(h0z, 0.0)
            hi0 = [consts.tile([128, NK, BL], bf16, name=f"hi0_{l}") for l in range(2)]
            lo0 = [consts.tile([128, NK, BL], bf16, name=f"lo0_{l}") for l in range(2)]
            for l in range(2):
                nc.vector.memset(hi0[l], 0.0)
                nc.vector.memset(lo0[l], 0.0)

            # ---- SBUF rings ----
            xm_ring = [consts.tile([F, 2, CHTOK], f32r, name=f"xm{i}") for i in range(3)]
            gxr = [[consts.tile([128, GROUP, NJ, BL], f32, name=f"gx{l}_{i}")
                    for i in range(2)] for l in range(2)]
            h1_ring = [consts.tile([128, NK, CHTOK], f32r, name=f"h1r{i}") for i in range(2)]
            h2_ring = [consts.tile([128, NK, CHTOK], f32r, name=f"h2r{i}") for i in range(2)]
            acc = consts.tile([128, NK, BL], f32)
            nc.vector.memset(acc, 0.0)
            lastx = consts.tile([128, NK, BL], f32)

            tc.strict_bb_all_engine_barrier()

            last_h = {0: (hi0[0], lo0[0]), 1: (hi0[1], lo0[1])}

            with tc.tile_pool(name="l0ps", bufs=2, space="PSUM") as l0ps, \
                 tc.tile_pool(name="l1ps", bufs=2, space="PSUM") as l1ps, \
                 tc.tile_pool(name="gps", bufs=2, space="PSUM") as gps, \
                 tc.tile_pool(name="eps", bufs=1, space="PSUM") as eps_p, \
                 tc.tile_pool(name="et", bufs=2) as et, \
                 tc.tile_pool(name="tmp", bufs=3) as tmp:

                def emit_xm_dma(c):
                    nc.sync.dma_start(
                        out=xm_ring[c % 3],
                        in_=xmT[:, ds(c * CHTOK, CHTOK)].rearrange(
                            "(k f) t -> f k t", k=2))

                def gx0_slices(c):
                    xm = xm_ring[c % 3]
                    ring = gxr[0][c % 2]
                    def mk(j):
                        def f():
                            ps = gps.tile([128, GROUP, BL], f32, tag="gps")
                            nc.tensor.matmul(ps, r32(w0_sb[:, 0, ts(j, 128)]),
                                             r32(xm[:, 0, :]), start=True, stop=False)
                            nc.tensor.matmul(ps, r32(w0_sb[:, 1, ts(j, 128)]),
                                             r32(xm[:, 1, :]), start=False, stop=True)
                            nc.scalar.activation(out=ring[:, :, j, :], in_=ps,
                                                 func=AF.Identity,
                                                 bias=gb_sb[0][:, j:j + 1])
                        return f
                    return [mk(j) for j in range(NJ)]

                def gx1_slices(g):
                    hsrc = h1_ring[g % 2]
                    ring = gxr[1][g % 2]
                    def mk(j):
                        def f():
                            ps = gps.tile([128, GROUP, BL], f32, tag="gps")
                            for k in range(NK):
                                nc.tensor.matmul(ps, r32(w1_sb[:, k, ts(j, 128)]),
                                                 r32(hsrc[:, k, :]),
                                                 start=(k == 0), stop=(k == NK - 1))
                            nc.scalar.activation(out=ring[:, :, j, :], in_=ps,
                                                 func=AF.Identity,
                                                 bias=gb_sb[1][:, j:j + 1])
                        return f
                    return [mk(j) for j in range(NJ)]

                def emit_step(layer, g, i):
                    ring = h1_ring if layer == 0 else h2_ring
                    gx = gxr[layer][g % 2]
                    whh = whh_sb[layer]
                    psp = l0ps if layer == 0 else l1ps
                    if g == 0 and i == 0:
                        hprev = h0z[:]
                    elif i == 0:
                        hprev = ring[(g - 1) % 2][:, :, (GROUP - 1) * BL:]
                    else:
                        hprev = ring[g % 2][:, :, (i - 1) * BL:i * BL]
                    if not (g == 0 and i == 0):
                        hprev = hprev.bitcast(f32)
                    hhi, hlo = last_h[layer]
                    ps = psp.tile([128, NJ, BL], f32, tag=f"ps{layer}")

                    def mmj(j):
                        for k in range(NK):
                            w = whh[:, j, k, :]
                            nc.tensor.matmul(ps[:, j, :], w, hhi[:, k, :],
                                             start=(k == 0),
                                             stop=(not USE_HILO and k == NK - 1))
                            if USE_HILO:
                                nc.tensor.matmul(ps[:, j, :], w, hlo[:, k, :],
                                                 start=False, stop=(k == NK - 1))
                    # r/z gate matmuls first; their add+sigmoid overlap the
                    # n-gate matmuls so the post-burst serial chain is shorter.
                    for j in range(8):
                        mmj(j)
                    rzp = tmp.tile([128, 8, BL], f32, tag=f"rzp{layer}")
                    nc.vector.tensor_add(rzp, ps[:, 0:8, :], gx[:, i, 0:8, :])
                    rz = tmp.tile([128, 8, BL], f32, tag=f"rz{layer}")
                    nc.scalar.activation(out=rz, in_=rzp, func=AF.Sigmoid)
                    for j in range(8, NJ):
                        mmj(j)
                    nb = tmp.tile([128, NK, BL], f32, tag=f"nb{layer}")
                    nc.vector.tensor_add(nb, ps[:, 8:12, :], bhn_sb[layer])
                    nh = tmp.tile([128, NK, BL], f32, tag=f"nh{layer}")
                    nc.vector.tensor_mul(nh, rz[:, 0:4, :], nb)
                    npre = tmp.tile([128, NK, BL], f32, tag=f"np{layer}")
                    nc.vector.tensor_add(npre, nh, gx[:, i, 8:12, :])
                    n = tmp.tile([128, NK, BL], f32, tag=f"n{layer}")
                    nc.scalar.activation(out=n, in_=npre, func=AF.Tanh)
                    d = tmp.tile([128, NK, BL], f32, tag=f"d{layer}")
                    nc.vector.tensor_sub(d, hprev, n)
                    zd = tmp.tile([128, NK, BL], f32, tag=f"zd{layer}")
                    nc.vector.tensor_mul(zd, rz[:, 4:8, :], d)
                    hn = ring[g % 2][:, :, i * BL:(i + 1) * BL]
                    with nc.allow_low_precision(reason="f32r keeps full fp32 bits; rounding happens at the consuming matmul"):
                        nc.vector.tensor_add(hn, n, zd)
                    hn = hn.bitcast(f32)
                    hi = tmp.tile([128, NK, BL], bf16, tag=f"hi{layer}")
                    nc.scalar.activation(out=hi, in_=hn, func=AF.Copy)
                    lo = None
                    if USE_HILO:
                        lo = tmp.tile([128, NK, BL], bf16, tag=f"lo{layer}")
                        nc.vector.tensor_sub(lo, hn, hi)
                    last_h[layer] = (hi, lo)

                def emit_e_chunk(g, is_last):
                    # LN stats + normalized pooling for h2 group g, straight
                    # from the SBUF ring (mean via ones/H matmul broadcast).
                    hr = h2_ring[g % 2]
                    ps_mu = eps_p.tile([128, CHTOK], f32, tag="mu")
                    for k in range(NK):
                        nc.tensor.matmul(ps_mu, onesH, hr[:, k, :],
                                         start=(k == 0), stop=(k == NK - 1))
                    sq = et.tile([128, NK, CHTOK], f32r, tag="sq")
                    nc.scalar.activation(out=sq, in_=hr.bitcast(f32), func=AF.Square)
                    ps_sq = eps_p.tile([128, CHTOK], f32, tag="sq")
                    for k in range(NK):
                        nc.tensor.matmul(ps_sq, onesH, sq[:, k, :],
                                         start=(k == 0), stop=(k == NK - 1))
                    mu2 = et.tile([128, CHTOK], f32, tag="mu2")
                    nc.scalar.activation(out=mu2, in_=ps_mu, func=AF.Square)
                    var = et.tile([128, CHTOK], f32, tag="var")
                    nc.vector.tensor_sub(var, ps_sq, mu2)
                    lnv = et.tile([128, CHTOK], f32, tag="lnv")
                    nc.scalar.activation(out=lnv, in_=var, func=AF.Ln, bias=eps_sb)
                    rs = et.tile([128, CHTOK], f32, tag="rs")
                    nc.scalar.activation(out=rs, in_=lnv, func=AF.Exp, scale=-0.5)
                    xh = et.tile([128, NK, CHTOK], f32, tag="xh")
                    mub = ps_mu.rearrange("p (k t) -> p k t", k=1).broadcast_to(
                        [128, NK, CHTOK])
                    nc.vector.tensor_sub(xh, hr.bitcast(f32), mub)
                    rsb = rs.rearrange("p (k t) -> p k t", k=1).broadcast_to(
                        [128, NK, CHTOK])
                    nc.vector.tensor_mul(xh, xh, rsb)
                    red = et.tile([128, NK, BL], f32, tag="red")
                    nc.vector.tensor_reduce(
                        red, xh.rearrange("p k (t b) -> p k b t", b=BL),
                        axis=mybir.AxisListType.X, op=AluOpType.add)
                    nc.vector.tensor_add(acc, acc, red)
                    if is_last:
                        nc.scalar.activation(out=lastx,
                                             in_=xh[:, :, (GROUP - 1) * BL:],
                                             func=AF.Copy)

                # ---- pipelined main loop ----
                emit_xm_dma(0)
                emit_xm_dma(1)
                for fn in gx0_slices(0):
                    fn()
                for g in range(NG + 2):
                    if g + 2 < NG:
                        emit_xm_dma(g + 2)
                    slices = []
                    if g + 1 < NG:
                        slices += gx0_slices(g + 1)
                    if 0 <= g - 1 < NG:
                        slices += gx1_slices(g - 1)
                    per = (len(slices) + GROUP - 1) // GROUP if slices else 0
                    si = 0
                    for i in range(GROUP):
                        if g < NG:
                            emit_step(0, g, i)
                        if 0 <= g - 2 < NG:
                            emit_step(1, g - 2, i)
                        for _ in range(per):
                            if si < len(slices):
                                slices[si]()
                                si += 1
                    while si < len(slices):
                        slices[si]()
                        si += 1
                    if 0 <= g - 2 < NG:
                        emit_e_chunk(g - 2, g - 2 == NG - 1)

            # ========= epilogue: pooled = acc/T + x̂(T-1); proj + GELU =========
            # (LN affine folded into wp/bp on host)
            with tc.tile_pool(name="fin", bufs=1) as fin, \
                 tc.tile_pool(name="fps", bufs=1, space="PSUM") as fps:
                po = fin.tile([128, NK, BL], f32r)
                with nc.allow_low_precision(reason="f32r keeps full fp32 bits"):
                    nc.vector.scalar_tensor_tensor(po, acc, 1.0 / T, lastx,
                                                   op0=AluOpType.mult,
                                                   op1=AluOpType.add)
                for jj in range(2):
                    psy = fps.tile([128, BL], f32, tag="psy")
                    for k in range(NK):
                        nc.tensor.matmul(psy, wp_sb[:, k, ts(jj, 128)],
                                         po[:, k, :],
                                         start=(k == 0), stop=(k == NK - 1))
                    yj = fin.tile([128, BL], f32, tag="yj")
                    nc.scalar.activation(out=yj, in_=psy,
                                         func=AF.Identity if SIM_MODE else AF.Gelu,
                                         bias=bp_sb[:, jj:jj + 1])
                    nc.sync.dma_start(out=out[jj], in_=yj)
    nc.finalize()
    return nc


# ---------------- host-side input prep ----------------

def prep_shared(W_ih0, W_hh0, b_ih0, b_hh0, W_ih1, W_hh1, b_ih1, b_hh1,
                ln_g, ln_b, W_proj, b_proj):
    def whh_tiles(W_hh):
        # [p, j, k, m] = W_hh^T[128k+p, 128j+m]
        w = np.ascontiguousarray(W_hh.T).reshape(NK, 128, NJ, 128)
        return np.ascontiguousarray(w.transpose(1, 2, 0, 3)).astype(ml_dtypes.bfloat16)

    def fold_bias(b_ih, b_hh):
        g = b_ih.copy()
        g[:2 * H] += b_hh[:2 * H]
        return np.ascontiguousarray(g.reshape(NJ, 128).T)  # [128, NJ]

    shared = {}
    # w0T[f, k, g] = W_ih0[g, k*F + f]
    w0 = np.ascontiguousarray(W_ih0.T)            # [130, 1536]
    shared["w0T"] = np.ascontiguousarray(w0.reshape(2, F, H3).transpose(1, 0, 2))
    # w1T[p, k, g] = W_ih1[g, 128k+p]
    w1 = np.ascontiguousarray(W_ih1.T)            # [512, 1536]
    shared["w1T"] = np.ascontiguousarray(w1.reshape(NK, 128, H3).transpose(1, 0, 2))
    shared["whh0"] = whh_tiles(W_hh0)
    shared["whh1"] = whh_tiles(W_hh1)
    shared["gb0"] = fold_bias(b_ih0, b_hh0)
    shared["gb1"] = fold_bias(b_ih1, b_hh1)
    shared["bhn0"] = np.ascontiguousarray(b_hh0[2 * H:].reshape(NK, 128).T)
    shared["bhn1"] = np.ascontiguousarray(b_hh1[2 * H:].reshape(NK, 128).T)
    # LN affine folded into proj: y = Wp@(x̂*g + b)*... -> (Wp*g)@p̂ + (bp + 2*Wp@b)
    Wg = W_proj * ln_g[None, :]
    bp2 = b_proj + 2.0 * (W_proj @ ln_b)
    # wpT[p, k, c] = Wg[c, 128k+p]
    shared["wpT"] = np.ascontiguousarray(Wg.T.reshape(NK, 128, 256).transpose(1, 0, 2))
    shared["bp"] = np.ascontiguousarray(bp2.reshape(2, 128).T)
    shared = {k: np.asarray(v, dtype=(ml_dtypes.bfloat16 if k.startswith("whh") else np.float32))
              for k, v in shared.items()}
    return shared


def prep_xmT(x_core, mask_core):
    # xmT[f, t*bl + b] = concat(x, mask)[b, t, f]
    xm = np.concatenate([x_core, mask_core.astype(np.float32)], axis=-1)  # [bl,T,2F]
    return np.ascontiguousarray(xm.transpose(2, 1, 0).reshape(2 * F, TB),
                                dtype=np.float32)


_CACHE = {}


def _enable_trace_support():
    """Profiling-only shim (used by test.py, not the graded path)."""
    import sys
    import types
    import concourse.bass_utils as bu
    bu.upload_artifacts = lambda tmpdir: "local://" + tmpdir
    try:
        from antenv.axon_hooks import get_axon_ntff_profile_hook  # noqa: F401
        return
    except ImportError:
        pass
    from trn_agent_boot.trn_boot import _ntff_profile_via_ctypes
    hook = _ntff_profile_via_ctypes("/opt/axon/libaxon_pjrt.so")
    mod = types.ModuleType("antenv.axon_hooks")
    mod.get_axon_ntff_profile_hook = lambda: hook
    mod.set_axon_ntff_profile_hook = lambda h: None
    sys.modules["antenv.axon_hooks"] = mod


def kernel(x, mask, W_ih0, W_hh0, b_ih0, b_hh0, W_ih1, W_hh1, b_ih1, b_hh1,
           ln_g, ln_b, W_proj, b_proj):
    from concourse.bass_utils import run_bass_kernel_spmd

    if "nc" not in _CACHE:
        _CACHE["nc"] = build_nc()
    nc = _CACHE["nc"]

    x = np.asarray(x, np.float32)
    mask = np.asarray(mask)
    shared = prep_shared(np.asarray(W_ih0, np.float32), np.asarray(W_hh0, np.float32),
                         np.asarray(b_ih0, np.float32), np.asarray(b_hh0, np.float32),
                         np.asarray(W_ih1, np.float32), np.asarray(W_hh1, np.float32),
                         np.asarray(b_ih1, np.float32), np.asarray(b_hh1, np.float32),
                         np.asarray(ln_g, np.float32), np.asarray(ln_b, np.float32),
                         np.asarray(W_proj, np.float32), np.asarray(b_proj, np.float32))
    in_maps = []
    for c in range(NCORES):
        m = dict(shared)
        m["xmT"] = prep_xmT(x[c * BL:(c + 1) * BL], mask[c * BL:(c + 1) * BL])
        in_maps.append(m)

    trace = os.environ.get("KERNEL_TRACE", "0") == "1"
    kw = {}
    if trace:
        _enable_trace_support()
        kw["tmpdir"] = os.environ.get("KERNEL_TRACE_DIR") or None
    res = run_bass_kernel_spmd(nc, in_maps, list(range(NCORES)), trace=trace, **kw)
    _CACHE["exec_time_ns"] = res.exec_time_ns
    if res.instructions_and_trace is not None:
        _CACHE["trace_path"] = res.instructions_and_trace[1]
    outs = []
    for c in range(NCORES):
        y = res.results[c]["out"]          # [2, 128, BL]
        outs.append(y.reshape(256, BL).T)  # [BL, 256]
    return np.ascontiguousarray(np.concatenate(outs, axis=0), dtype=np.float32)


# revision 31
# speedup vs baseline: 1.0096x; 1.0096x over previous
"""Trainium2 Bass kernel for nn_DataONEEncoder (2-layer GRU + LN + pool + proj + GELU).

Data-parallel over batch: B=256 -> 32 per core on 8 NeuronCores, no collectives.

v2: single software-pipelined main loop. Layer-0 scan steps, layer-1 scan steps
(lagging 2 groups of 8 steps), and the gx0/gx1 input-projection GEMMs (computed
as j-slices sprinkled between scan steps) all interleave so the PE never idles
long enough to stall on the serial gate chain or re-throttle (HAM). gx and h
intermediates live in SBUF rings; only xm (in) and h2 (out, for the LN phase)
touch DRAM.

Phase E (LayerNorm + pool + proj + GELU): LN affine (g, b) is folded into
W_proj/b_proj on the host; rsqrt computed as exp(-0.5*ln(var+eps)) on the
scalar engine with 128-partition-parallel stats (mean via ones/H matmul
broadcast), avoiding the 1-partition DVE reciprocal.
"""

import os
import numpy as np
import ml_dtypes

import concourse.bass as bass
from concourse import bacc
import concourse.mybir as mybir
import concourse.tile as tile
from concourse.alu_op_type import AluOpType
from concourse.bass import ts, ds

B, T, F, H = 256, 512, 65, 512
NCORES = 8
BL = B // NCORES          # 32 batch per core
H3 = 3 * H                # 1536
NJ = H3 // 128            # 12 output tiles of the gate dim
NK = H // 128             # 4 contraction tiles of the hidden dim
TB = T * BL               # tokens per core
EPS = 1e-5
GROUP = 8                 # scan steps per pipeline group
NG = T // GROUP           # 64 groups
CHTOK = GROUP * BL        # 256 tokens per gx GEMM chunk

f32 = mybir.dt.float32
f32r = mybir.dt.float32r
bf16 = mybir.dt.bfloat16
AF = mybir.ActivationFunctionType

USE_HILO = os.environ.get("KERNEL_HILO", "0") == "1"
SIM_MODE = os.environ.get("KERNEL_SIM", "0") == "1"   # CoreSim lacks Gelu


def r32(ap):
    return ap.bitcast(f32r)


def build_nc():
    nc = bacc.Bacc()

    # ---- external inputs (host pre-laid-out, see kernel()) ----
    xmT = nc.declare_dram_parameter("xmT", [2 * F, TB], f32r, isOutput=False)    # [f, (t,b)]
    w0T = nc.declare_dram_parameter("w0T", [F, 2, H3], f32r, isOutput=False)     # [f, k(x|m), g]
    w1T = nc.declare_dram_parameter("w1T", [128, NK, H3], f32r, isOutput=False)  # [p, k, g]
    whh0 = nc.declare_dram_parameter("whh0", [128, NJ, NK, 128], bf16, isOutput=False)
    whh1 = nc.declare_dram_parameter("whh1", [128, NJ, NK, 128], bf16, isOutput=False)
    gb0 = nc.declare_dram_parameter("gb0", [128, NJ], f32, isOutput=False)       # folded bias
    gb1 = nc.declare_dram_parameter("gb1", [128, NJ], f32, isOutput=False)
    bhn0 = nc.declare_dram_parameter("bhn0", [128, NK], f32, isOutput=False)     # b_hh n-gate
    bhn1 = nc.declare_dram_parameter("bhn1", [128, NK], f32, isOutput=False)
    wpT = nc.declare_dram_parameter("wpT", [128, NK, 256], f32r, isOutput=False)  # ln_g folded
    bp = nc.declare_dram_parameter("bp", [128, 2], f32, isOutput=False)           # 2*Wp@ln_b folded
    out = nc.declare_dram_parameter("out", [2, 128, BL], f32, isOutput=True)

    with tile.TileContext(nc) as tc:
        with tc.tile_pool(name="dram", bufs=1, space="DRAM") as dram, \
             tc.tile_pool(name="consts", bufs=1) as consts:

            # h2 kept in DRAM for the (separate) LN/pool phase.
            h2T = dram.tile([NK, 128, TB], f32r)

            # ---- load constants to SBUF ----
            w0_sb = consts.tile([F, 2, H3], f32r)
            nc.sync.dma_start(out=w0_sb, in_=w0T[:])
            w1_sb = consts.tile([128, NK, H3], f32r)
            nc.sync.dma_start(out=w1_sb, in_=w1T[:])
            whh_sb = [consts.tile([128, NJ, NK, 128], bf16, name=f"whh{i}_sb") for i in range(2)]
            nc.sync.dma_start(out=whh_sb[0], in_=whh0[:])
            nc.sync.dma_start(out=whh_sb[1], in_=whh1[:])
            gb_sb = [consts.tile([128, NJ], f32, name=f"gb{i}_sb") for i in range(2)]
            nc.sync.dma_start(out=gb_sb[0], in_=gb0[:])
            nc.sync.dma_start(out=gb_sb[1], in_=gb1[:])
            # broadcast b_hh(n) over batch -> [128, NK, BL]
            bhn_small = [consts.tile([128, NK], f32, name=f"bhn{i}_sm") for i in range(2)]
            bhn_sb = [consts.tile([128, NK, BL], f32, name=f"bhn{i}_sb") for i in range(2)]
            for i, srcp in enumerate((bhn0, bhn1)):
                nc.sync.dma_start(out=bhn_small[i], in_=srcp[:])
                nc.vector.tensor_copy(out=bhn_sb[i],
                                      in_=bhn_small[i].to_broadcast([128, NK, BL]))
            wp_sb = consts.tile([128, NK, 256], f32r)
            nc.sync.dma_start(out=wp_sb, in_=wpT[:])
            bp_sb = consts.tile([128, 2], f32)
            nc.sync.dma_start(out=bp_sb, in_=bp[:])
            ones_stage = consts.tile([128, 128], f32)
            nc.vector.memset(ones_stage, 1.0 / H)
            onesH = consts.tile([128, 128], f32r)   # lhsT for partition-mean bcast
            nc.vector.tensor_copy(out=onesH, in_=ones_stage)
            eps_sb = consts.tile([128, 1], f32)
            nc.vector.memset(eps_sb, EPS)
            h0z = consts.tile([128, NK, BL], f32)
            nc.vector.memset(h0z, 0.0)
            hi0 = [consts.tile([128, NK, BL], bf16, name=f"hi0_{l}") for l in range(2)]
            lo0 = [consts.tile([128, NK, BL], bf16, name=f"lo0_{l}") for l in range(2)]
            for l in range(2):
                nc.vector.memset(hi0[l], 0.0)
                nc.vector.memset(lo0[l], 0.0)

            # ---- SBUF rings ----
            xm_ring = [consts.tile([F, 2, CHTOK], f32r, name=f"xm{i}") for i in range(3)]
            gxr = [[consts.tile([128, GROUP, NJ, BL], f32, name=f"gx{l}_{i}")
                    for i in range(2)] for l in range(2)]
            h1_ring = [consts.tile([128, NK, CHTOK], f32r, name=f"h1r{i}") for i in range(2)]
            h2_ring = [consts.tile([128, NK, CHTOK], f32, name=f"h2r{i}") for i in range(2)]

            tc.strict_bb_all_engine_barrier()

            last_h = {0: (hi0[0], lo0[0]), 1: (hi0[1], lo0[1])}

            with tc.tile_pool(name="l0ps", bufs=2, space="PSUM") as l0ps, \
                 tc.tile_pool(name="l1ps", bufs=2, space="PSUM") as l1ps, \
                 tc.tile_pool(name="gps", bufs=4, space="PSUM") as gps, \
                 tc.tile_pool(name="tmp", bufs=3) as tmp:

                def emit_xm_dma(c):
                    nc.sync.dma_start(
                        out=xm_ring[c % 3],
                        in_=xmT[:, ds(c * CHTOK, CHTOK)].rearrange(
                            "(k f) t -> f k t", k=2))

                def gx0_slices(c):
                    xm = xm_ring[c % 3]
                    ring = gxr[0][c % 2]
                    def mk(j):
                        def f():
                            ps = gps.tile([128, GROUP, BL], f32, tag="gps")
                            nc.tensor.matmul(ps, r32(w0_sb[:, 0, ts(j, 128)]),
                                             r32(xm[:, 0, :]), start=True, stop=False)
                            nc.tensor.matmul(ps, r32(w0_sb[:, 1, ts(j, 128)]),
                                             r32(xm[:, 1, :]), start=False, stop=True)
                            nc.scalar.activation(out=ring[:, :, j, :], in_=ps,
                                                 func=AF.Identity,
                                                 bias=gb_sb[0][:, j:j + 1])
                        return f
                    return [mk(j) for j in range(NJ)]

                def gx1_slices(g):
                    hsrc = h1_ring[g % 2]
                    ring = gxr[1][g % 2]
                    def mk(j):
                        def f():
                            ps = gps.tile([128, GROUP, BL], f32, tag="gps")
                            for k in range(NK):
                                nc.tensor.matmul(ps, r32(w1_sb[:, k, ts(j, 128)]),
                                                 r32(hsrc[:, k, :]),
                                                 start=(k == 0), stop=(k == NK - 1))
                            nc.scalar.activation(out=ring[:, :, j, :], in_=ps,
                                                 func=AF.Identity,
                                                 bias=gb_sb[1][:, j:j + 1])
                        return f
                    return [mk(j) for j in range(NJ)]

                def emit_step(layer, g, i):
                    ring = h1_ring if layer == 0 else h2_ring
                    gx = gxr[layer][g % 2]
                    whh = whh_sb[layer]
                    psp = l0ps if layer == 0 else l1ps
                    if g == 0 and i == 0:
                        hprev = h0z[:]
                    elif i == 0:
                        hprev = ring[(g - 1) % 2][:, :, (GROUP - 1) * BL:]
                    else:
                        hprev = ring[g % 2][:, :, (i - 1) * BL:i * BL]
                    if layer == 0 and not (g == 0 and i == 0):
                        hprev = hprev.bitcast(f32)
                    hhi, hlo = last_h[layer]
                    ps = psp.tile([128, NJ, BL], f32, tag=f"ps{layer}")

                    def mmj(j):
                        for k in range(NK):
                            w = whh[:, j, k, :]
                            nc.tensor.matmul(ps[:, j, :], w, hhi[:, k, :],
                                             start=(k == 0),
                                             stop=(not USE_HILO and k == NK - 1))
                            if USE_HILO:
                                nc.tensor.matmul(ps[:, j, :], w, hlo[:, k, :],
                                                 start=False, stop=(k == NK - 1))
                    # r/z gate matmuls first; their add+sigmoid overlap the
                    # n-gate matmuls so the post-burst serial chain is shorter.
                    for j in range(8):
                        mmj(j)
                    rzp = tmp.tile([128, 8, BL], f32, tag=f"rzp{layer}")
                    nc.vector.tensor_add(rzp, ps[:, 0:8, :], gx[:, i, 0:8, :])
                    rz = tmp.tile([128, 8, BL], f32, tag=f"rz{layer}")
                    nc.scalar.activation(out=rz, in_=rzp, func=AF.Sigmoid)
                    for j in range(8, NJ):
                        mmj(j)
                    nb = tmp.tile([128, NK, BL], f32, tag=f"nb{layer}")
                    nc.vector.tensor_add(nb, ps[:, 8:12, :], bhn_sb[layer])
                    nh = tmp.tile([128, NK, BL], f32, tag=f"nh{layer}")
                    nc.vector.tensor_mul(nh, rz[:, 0:4, :], nb)
                    npre = tmp.tile([128, NK, BL], f32, tag=f"np{layer}")
                    nc.vector.tensor_add(npre, nh, gx[:, i, 8:12, :])
                    n = tmp.tile([128, NK, BL], f32, tag=f"n{layer}")
                    nc.scalar.activation(out=n, in_=npre, func=AF.Tanh)
                    d = tmp.tile([128, NK, BL], f32, tag=f"d{layer}")
                    nc.vector.tensor_sub(d, hprev, n)
                    zd = tmp.tile([128, NK, BL], f32, tag=f"zd{layer}")
                    nc.vector.tensor_mul(zd, rz[:, 4:8, :], d)
                    hn = ring[g % 2][:, :, i * BL:(i + 1) * BL]
                    if layer == 0:
                        with nc.allow_low_precision(reason="f32r keeps full fp32 bits; rounding happens at the gx1 matmul"):
                            nc.vector.tensor_add(hn, n, zd)
                        hn = hn.bitcast(f32)
                    else:
                        nc.vector.tensor_add(hn, n, zd)
                    hi = tmp.tile([128, NK, BL], bf16, tag=f"hi{layer}")
                    nc.scalar.activation(out=hi, in_=hn, func=AF.Copy)
                    lo = None
                    if USE_HILO:
                        lo = tmp.tile([128, NK, BL], bf16, tag=f"lo{layer}")
                        nc.vector.tensor_sub(lo, hn, hi)
                    last_h[layer] = (hi, lo)

                def emit_h2_dma(g):
                    nc.sync.dma_start(
                        out=h2T[:, :, g * CHTOK:(g + 1) * CHTOK].rearrange(
                            "k p m -> p k m"),
                        in_=r32(h2_ring[g % 2]))

                # ---- pipelined main loop ----
                emit_xm_dma(0)
                emit_xm_dma(1)
                for fn in gx0_slices(0):
                    fn()
                for g in range(NG + 2):
                    if g + 2 < NG:
                        emit_xm_dma(g + 2)
                    slices = []
                    if g + 1 < NG:
                        slices += gx0_slices(g + 1)
                    if 0 <= g - 1 < NG:
                        slices += gx1_slices(g - 1)
                    per = (len(slices) + GROUP - 1) // GROUP if slices else 0
                    si = 0
                    for i in range(GROUP):
                        if g < NG:
                            emit_step(0, g, i)
                        if 0 <= g - 2 < NG:
                            emit_step(1, g - 2, i)
                        for _ in range(per):
                            if si < len(slices):
                                slices[si]()
                                si += 1
                    while si < len(slices):
                        slices[si]()
                        si += 1
                    if 0 <= g - 2 < NG:
                        emit_h2_dma(g - 2)

            # ================= Phase E: LN + pool + proj + GELU =================
            NTOK = 256
            nchunks = TB // NTOK
            SPC = NTOK // BL    # steps per chunk
            with tc.tile_pool(name="e_in", bufs=3) as e_in, \
                 tc.tile_pool(name="e_sq", bufs=2) as e_sq, \
                 tc.tile_pool(name="e_t", bufs=3) as e_t, \
                 tc.tile_pool(name="e_acc", bufs=1) as e_acc, \
                 tc.tile_pool(name="e_ps", bufs=2, space="PSUM") as e_ps:
                acc = e_acc.tile([128, NK, BL], f32)
                nc.vector.memset(acc, 0.0)
                lastx = e_acc.tile([128, NK, BL], f32)
                for c in range(nchunks):
                    tok = ds(c * NTOK, NTOK)
                    hch = e_in.tile([128, NK, NTOK], f32r, tag="hch")
                    for k in range(NK):
                        nc.sync.dma_start(out=hch[:, k, :], in_=h2T[k, :, tok])
                    ps_mu = e_ps.tile([128, NTOK], f32, tag="mu")
                    for k in range(NK):
                        nc.tensor.matmul(ps_mu, onesH, hch[:, k, :],
                                         start=(k == 0), stop=(k == NK - 1))
                    sq = e_sq.tile([128, NK, NTOK], f32r, tag="sq")
                    nc.scalar.activation(out=sq, in_=hch.bitcast(f32), func=AF.Square)
                    ps_sq = e_ps.tile([128, NTOK], f32, tag="sq")
                    for k in range(NK):
                        nc.tensor.matmul(ps_sq, onesH, sq[:, k, :],
                                         start=(k == 0), stop=(k == NK - 1))
                    mu2 = e_t.tile([128, NTOK], f32, tag="mu2")
                    nc.scalar.activation(out=mu2, in_=ps_mu, func=AF.Square)
                    var = e_t.tile([128, NTOK], f32, tag="var")
                    nc.vector.tensor_sub(var, ps_sq, mu2)
                    lnv = e_t.tile([128, NTOK], f32, tag="lnv")
                    nc.scalar.activation(out=lnv, in_=var, func=AF.Ln, bias=eps_sb)
                    rs = e_t.tile([128, NTOK], f32, tag="rs")
                    nc.scalar.activation(out=rs, in_=lnv, func=AF.Exp, scale=-0.5)
                    # x̂ written back in place of hch (mu/rs broadcast over k)
                    mub = ps_mu.rearrange("p (k t) -> p k t", k=1).broadcast_to(
                        [128, NK, NTOK])
                    cen = e_t.tile([128, NK, NTOK], f32, tag="cen")
                    nc.vector.tensor_sub(cen, hch.bitcast(f32), mub)
                    rsb = rs.rearrange("p (k t) -> p k t", k=1).broadcast_to(
                        [128, NK, NTOK])
                    with nc.allow_low_precision(reason="f32r keeps full fp32 bits"):
                        nc.vector.tensor_mul(hch, cen, rsb)
                    red = e_t.tile([128, NK, BL], f32, tag="red")
                    nc.vector.tensor_reduce(
                        red, hch.bitcast(f32).rearrange("p k (t b) -> p k b t", b=BL),
                        axis=mybir.AxisListType.X, op=AluOpType.add)
                    nc.vector.tensor_add(acc, acc, red)
                    if c == nchunks - 1:
                        nc.scalar.activation(out=lastx,
                                             in_=hch.bitcast(f32)[:, :, (SPC - 1) * BL:],
                                             func=AF.Copy)
                # pooled = acc/T + x̂(T-1)   (affine folded into wp/bp on host)
                po = e_acc.tile([128, NK, BL], f32r)
                with nc.allow_low_precision(reason="f32r keeps full fp32 bits"):
                    nc.vector.scalar_tensor_tensor(po, acc, 1.0 / T, lastx,
                                                   op0=AluOpType.mult,
                                                   op1=AluOpType.add)
                for jj in range(2):
                    psy = e_ps.tile([128, BL], f32, tag="psy")
                    for k in range(NK):
                        nc.tensor.matmul(psy, wp_sb[:, k, ts(jj, 128)],
                                         po[:, k, :],
                                         start=(k == 0), stop=(k == NK - 1))
                    yj = e_t.tile([128, BL], f32, tag="yj")
                    nc.scalar.activation(out=yj, in_=psy,
                                         func=AF.Identity if SIM_MODE else AF.Gelu,
                                         bias=bp_sb[:, jj:jj + 1])
                    nc.sync.dma_start(out=out[jj], in_=yj)
    nc.finalize()
    return nc


# ---------------- host-side input prep ----------------

def prep_shared(W_ih0, W_hh0, b_ih0, b_hh0, W_ih1, W_hh1, b_ih1, b_hh1,
                ln_g, ln_b, W_proj, b_proj):
    def whh_tiles(W_hh):
        # [p, j, k, m] = W_hh^T[128k+p, 128j+m]
        w = np.ascontiguousarray(W_hh.T).reshape(NK, 128, NJ, 128)
        return np.ascontiguousarray(w.transpose(1, 2, 0, 3)).astype(ml_dtypes.bfloat16)

    def fold_bias(b_ih, b_hh):
        g = b_ih.copy()
        g[:2 * H] += b_hh[:2 * H]
        return np.ascontiguousarray(g.reshape(NJ, 128).T)  # [128, NJ]

    shared = {}
    # w0T[f, k, g] = W_ih0[g, k*F + f]
    w0 = np.ascontiguousarray(W_ih0.T)            # [130, 1536]
    shared["w0T"] = np.ascontiguousarray(w0.reshape(2, F, H3).transpose(1, 0, 2))
    # w1T[p, k, g] = W_ih1[g, 128k+p]
    w1 = np.ascontiguousarray(W_ih1.T)            # [512, 1536]
    shared["w1T"] = np.ascontiguousarray(w1.reshape(NK, 128, H3).transpose(1, 0, 2))
    shared["whh0"] = whh_tiles(W_hh0)
    shared["whh1"] = whh_tiles(W_hh1)
    shared["gb0"] = fold_bias(b_ih0, b_hh0)
    shared["gb1"] = fold_bias(b_ih1, b_hh1)
    shared["bhn0"] = np.ascontiguousarray(b_hh0[2 * H:].reshape(NK, 128).T)
    shared["bhn1"] = np.ascontiguousarray(b_hh1[2 * H:].reshape(NK, 128).T)
    # LN affine folded into proj: y = Wp@(x̂*g + b)*... -> (Wp*g)@p̂ + (bp + 2*Wp@b)
    Wg = W_proj * ln_g[None, :]
    bp2 = b_proj + 2.0 * (W_proj @ ln_b)
    # wpT[p, k, c] = Wg[c, 128k+p]
    shared["wpT"] = np.ascontiguousarray(Wg.T.reshape(NK, 128, 256).transpose(1, 0, 2))
    shared["bp"] = np.ascontiguousarray(bp2.reshape(2, 128).T)
    shared = {k: np.asarray(v, dtype=(ml_dtypes.bfloat16 if k.startswith("whh") else np.float32))
              for k, v in shared.items()}
    return shared


def prep_xmT(x_core, mask_core):
    # xmT[f, t*bl + b] = concat(x, mask)[b, t, f]
    xm = np.concatenate([x_core, mask_core.astype(np.float32)], axis=-1)  # [bl,T,2F]
    return np.ascontiguousarray(xm.transpose(2, 1, 0).reshape(2 * F, TB),
                                dtype=np.float32)


_CACHE = {}


def _enable_trace_support():
    """Profiling-only shim (used by test.py, not the graded path)."""
    import sys
    import types
    import concourse.bass_utils as bu
    bu.upload_artifacts = lambda tmpdir: "local://" + tmpdir
    try:
        from antenv.axon_hooks import get_axon_ntff_profile_hook  # noqa: F401
        return
    except ImportError:
        pass
    from trn_agent_boot.trn_boot import _ntff_profile_via_ctypes
    hook = _ntff_profile_via_ctypes("/opt/axon/libaxon_pjrt.so")
    mod = types.ModuleType("antenv.axon_hooks")
    mod.get_axon_ntff_profile_hook = lambda: hook
    mod.set_axon_ntff_profile_hook = lambda h: None
    sys.modules["antenv.axon_hooks"] = mod


def kernel(x, mask, W_ih0, W_hh0, b_ih0, b_hh0, W_ih1, W_hh1, b_ih1, b_hh1,
           ln_g, ln_b, W_proj, b_proj):
    from concourse.bass_utils import run_bass_kernel_spmd

    if "nc" not in _CACHE:
        _CACHE["nc"] = build_nc()
    nc = _CACHE["nc"]

    x = np.asarray(x, np.float32)
    mask = np.asarray(mask)
    shared = prep_shared(np.asarray(W_ih0, np.float32), np.asarray(W_hh0, np.float32),
                         np.asarray(b_ih0, np.float32), np.asarray(b_hh0, np.float32),
                         np.asarray(W_ih1, np.float32), np.asarray(W_hh1, np.float32),
                         np.asarray(b_ih1, np.float32), np.asarray(b_hh1, np.float32),
                         np.asarray(ln_g, np.float32), np.asarray(ln_b, np.float32),
                         np.asarray(W_proj, np.float32), np.asarray(b_proj, np.float32))
    in_maps = []
    for c in range(NCORES):
        m = dict(shared)
        m["xmT"] = prep_xmT(x[c * BL:(c + 1) * BL], mask[c * BL:(c + 1) * BL])
        in_maps.append(m)

    trace = os.environ.get("KERNEL_TRACE", "0") == "1"
    kw = {}
    if trace:
        _enable_trace_support()
        kw["tmpdir"] = os.environ.get("KERNEL_TRACE_DIR") or None
    res = run_bass_kernel_spmd(nc, in_maps, list(range(NCORES)), trace=trace, **kw)
    _CACHE["exec_time_ns"] = res.exec_time_ns
    if res.instructions_and_trace is not None:
        _CACHE["trace_path"] = res.instructions_and_trace[1]
    outs = []
    for c in range(NCORES):
        y = res.results[c]["out"]          # [2, 128, BL]
        outs.append(y.reshape(256, BL).T)  # [BL, 256]
    return np.ascontiguousarray(np.concatenate(outs, axis=0), dtype=np.float32)
